# revision 12
# baseline (speedup 1.0000x reference)
"""MACE-style GNN message passing on 8 Trainium2 NeuronCores.

Only the l=0 (scalar) channel of the reference reaches the output, so the
network collapses algebraically: per edge, the radial MLP's last hidden
t3 (64) is dotted with a per-(sender-species, receiver-species) vector
Gamma[s,z] = W4_0 @ (hu[s] * delta[z]), where hu = w_embed@w_up and
delta[z] folds w_lin[0], w_sym[0], w_lin2[0] and w_readout.  Node energy
is then ae[z]+beta[z] + (1/16) * scatter_sum(eps_e).

Device pipeline (bf16 matmuls, fp32 geometry):
  - batched DVE geometry for all edges: r via bit-trick rsqrt, envelope,
    bessel sine args; one ACT Sin; ef = sh*w (bf16)
  - per 1024-edge block: PE transpose ef -> [64 feat, e], 3-layer MLP
    (silu on ACT), PE transpose t3 -> [e, h], product with gathered
    Gamma rows (DVE/GPSIMD alternating), and a per-128-edge-subtile
    scatter matmul (one-hot stationary) accumulating msg[128 nodes, 64]
    per node tile in PSUM
  - epilogue: reduce h on DVE, scale + per-species constant, DMA out

Sharding: receivers range-partitioned (1000 nodes/core); per (core,
node-tile) edge groups padded to a uniform SEG subtiles of 128 so all
cores run one SPMD program.  Edges with r >= r_max are dropped on host.
"""

import sys
import numpy as np

sys.path.insert(0, "/opt/trn_rl_repo")

import ml_dtypes

BF16 = ml_dtypes.bfloat16

R_MAX = 5.0
EPS = 1e-9
AVG = 16.0
N_NODES = 8000
Z = 10
K = 128
NB = 8
NCORES = 8
NPC = N_NODES // NCORES       # nodes per core (1000)
NT = 8                        # node tiles per core (128 nodes each)

TRACE = False
LAST_RESULTS = None

_prog_cache = {}


def _build_program(SEG):
    """SPMD Bass program; SEG = 128-edge subtiles per 128-node tile."""
    from concourse import bass, bacc, mybir
    from concourse.tile import TileContext
    from contextlib import ExitStack

    f32 = mybir.dt.float32
    bf16 = mybir.dt.bfloat16
    i32 = mybir.dt.int32
    AF = mybir.ActivationFunctionType
    OP = mybir.AluOpType
    PSUM = bass.MemorySpace.PSUM

    S = NT * SEG              # total subtiles per core
    NBLK = S // 8             # 1024-edge blocks
    S3 = 3 * S
    S8 = 8 * S

    nc = bacc.Bacc(None, target_bir_lowering=False)

    ve_d = nc.dram_tensor("ve", [128, S3], f32, kind="ExternalInput")
    g_d = nc.dram_tensor("gtab", [NBLK, 128, 512], bf16, kind="ExternalInput")
    ohr_d = nc.dram_tensor("ohr", [NBLK, 128, 1024], bf16, kind="ExternalInput")
    cf_d = nc.dram_tensor("constf", [128, 18], f32, kind="ExternalInput")
    cb_d = nc.dram_tensor("constb", [128, 896], bf16, kind="ExternalInput")
    out_d = nc.dram_tensor("out", [128, 8], f32, kind="ExternalOutput")

    with TileContext(nc) as tc:
        with ExitStack() as stack:
            cp = stack.enter_context(tc.tile_pool(name="const", bufs=1))
            geo = stack.enter_context(tc.tile_pool(name="geo", bufs=1))
            efsp = stack.enter_context(tc.tile_pool(name="efsp", bufs=3))
            gp = stack.enter_context(tc.tile_pool(name="gp", bufs=3))
            ohp = stack.enter_context(tc.tile_pool(name="ohp", bufs=3))
            tp = stack.enter_context(tc.tile_pool(name="tp", bufs=6))
            qp = stack.enter_context(tc.tile_pool(name="qp", bufs=3))
            outp = stack.enter_context(tc.tile_pool(name="outp", bufs=1))
            pefp = stack.enter_context(tc.tile_pool(name="pefp", bufs=2, space=PSUM))
            pml = stack.enter_context(tc.tile_pool(name="pml", bufs=3, space=PSUM))
            pq3 = stack.enter_context(tc.tile_pool(name="pq3", bufs=2, space=PSUM))
            pmsg = stack.enter_context(tc.tile_pool(name="pmsg", bufs=1, space=PSUM))

            # ---- constants ----
            CTF = cp.tile([128, 18], f32)
            nc.sync.dma_start(CTF[:], cf_d[:])
            CTB = cp.tile([128, 896], bf16)
            nc.sync.dma_start(CTB[:], cb_d[:])
            CB8 = CTF[:, 0:8]
            CNODE = CTF[:, 8:16]
            ONEI = CTF[:, 16:17].bitcast(i32)
            MAGIC = CTF[:, 17:18].bitcast(i32)
            W1P = [CTB[0:64, 128 * j:128 * j + 128] for j in range(4)]
            W2BD = CTB[:, 512:640]
            W3BD = CTB[:, 640:768]
            I128 = CTB[:, 768:896]

            VE = geo.tile([128, S3], f32)
            nc.sync.dma_start(VE[:], ve_d[:])

            # geometry scratch
            SQ = geo.tile([128, S3], f32)
            SC = geo.tile([128, 8 * S], f32)   # per-subtile scalars, slices of S
            TH = geo.tile([128, S8], f32)
            KI = geo.tile([128, S8], i32)
            KF = geo.tile([128, S8], f32)
            SA = geo.tile([128, S8], f32)
            GT = geo.tile([128, S8], f32)
            SH = geo.tile([128, S8], f32)
            EFB = geo.tile([128, S8], bf16)

            tc.strict_bb_all_engine_barrier()

            def sl(i):
                return SC[:, i * S:(i + 1) * S]

            ss, y, h2, q2, r_, x, t1_, t2_ = (sl(i) for i in range(8))

            # squared distance
            nc.vector.tensor_tensor(SQ[:], VE[:], VE[:], OP.mult)
            nc.vector.tensor_reduce(
                ss, SQ[:].rearrange("p (s c) -> p s c", c=3),
                mybir.AxisListType.X, OP.add)
            nc.vector.tensor_scalar(ss, ss, EPS, None, OP.add)
            # rsqrt via bit trick + 2 Newton steps
            y_i = y.bitcast(i32)
            nc.vector.tensor_tensor(
                y_i, ss.bitcast(i32),
                ONEI.broadcast_to([128, S]), OP.arith_shift_right)
            nc.vector.tensor_tensor(
                y_i, MAGIC.broadcast_to([128, S]), y_i, OP.subtract)
            nc.vector.tensor_scalar(h2, ss, 0.5, None, OP.mult)
            for _ in range(2):
                nc.vector.tensor_tensor(q2, y, y, OP.mult)
                nc.vector.tensor_tensor(q2, q2, h2, OP.mult)
                nc.vector.tensor_scalar(q2, q2, -1.0, 1.5, OP.mult, OP.add)
                nc.vector.tensor_tensor(y, y, q2, OP.mult)
            nc.vector.tensor_tensor(r_, ss, y, OP.mult)     # r = sqrt(ss)
            # envelope: env = 1 + x^5*(-21 + x*(35 - 15x)) for x<1
            nc.vector.tensor_scalar(x, r_, 1.0 / R_MAX, None, OP.mult)
            nc.vector.tensor_tensor(t1_, x, x, OP.mult)          # x2
            nc.vector.tensor_tensor(t1_, t1_, t1_, OP.mult)      # x4
            nc.vector.tensor_tensor(t1_, t1_, x, OP.mult)        # x5
            nc.vector.tensor_scalar(t2_, x, -15.0, 35.0, OP.mult, OP.add)
            nc.vector.tensor_tensor(t2_, t2_, x, OP.mult)
            nc.vector.scalar_tensor_tensor(
                t1_, t2_, -21.0, t1_, OP.add, OP.mult)           # e1
            nc.vector.tensor_scalar(t2_, x, 1.0, None, OP.is_lt)  # mask
            nc.vector.scalar_tensor_tensor(
                t1_, t1_, 1.0, t2_, OP.add, OP.mult)             # env
            # w = env * sqrt(2/R) * rinv
            nc.vector.scalar_tensor_tensor(
                t2_, t1_, float(np.sqrt(2.0 / R_MAX)), y, OP.mult, OP.mult)
            w_ = t2_

            # bessel sine args, range-reduced to [-0.5, 0.5], scale 2pi in ACT
            th3 = TH[:].rearrange("p (s b) -> p s b", b=8)
            nc.vector.tensor_tensor(
                th3, CB8.unsqueeze(1).broadcast_to([128, S, 8]),
                r_.unsqueeze(2).broadcast_to([128, S, 8]), OP.mult)
            nc.vector.tensor_copy(KI[:], TH[:])
            nc.vector.tensor_copy(KF[:], KI[:])
            nc.vector.tensor_tensor(SA[:], TH[:], KF[:], OP.subtract)
            nc.vector.tensor_scalar(GT[:], SA[:], 0.5, None, OP.is_gt)
            nc.vector.scalar_tensor_tensor(
                SA[:], GT[:], -1.0, SA[:], OP.mult, OP.add)
            nc.scalar.activation(SH[:], SA[:], AF.Sin, scale=float(2 * np.pi))
            nc.vector.tensor_tensor(
                EFB[:].rearrange("p (s b) -> p s b", b=8),
                SH[:].rearrange("p (s b) -> p s b", b=8),
                w_.unsqueeze(2).broadcast_to([128, S, 8]), OP.mult)

            # ---- per-block MLP + scatter ----
            MSG = pmsg.tile([128, 512], f32, tag="msg")
            for t in range(NBLK):
                Gt = gp.tile([128, 512], bf16, tag="gt")
                nc.sync.dma_start(Gt[:], g_d[t])
                OHRt = ohp.tile([128, 1024], bf16, tag="ohr")
                nc.sync.dma_start(OHRt[:], ohr_d[t])

                pef = pefp.tile([64, 128], bf16, tag="pef")
                nc.tensor.transpose(pef[:], EFB[:, 64 * t:64 * t + 64], I128)
                efs = efsp.tile([64, 128], bf16, tag="efs")
                nc.vector.tensor_copy(efs[:], pef[:])

                p1 = pml.tile([128, 512], f32, tag="pml")
                for j in range(4):
                    nc.tensor.matmul(p1[:, 128 * j:128 * j + 128], W1P[j],
                                     efs[:], start=True, stop=True)
                t1 = tp.tile([128, 512], bf16, tag="t1")
                nc.scalar.activation(t1[:], p1[:], AF.Silu)
                p2 = pml.tile([128, 512], f32, tag="pml")
                nc.tensor.matmul(p2[:], W2BD, t1[:], start=True, stop=True)
                t2 = tp.tile([128, 512], bf16, tag="t2")
                nc.scalar.activation(t2[:], p2[:], AF.Silu)
                p3 = pml.tile([128, 512], f32, tag="pml")
                nc.tensor.matmul(p3[:], W3BD, t2[:], start=True, stop=True)
                t3 = tp.tile([128, 512], bf16, tag="t3")
                nc.scalar.activation(t3[:], p3[:], AF.Silu)

                q3 = pq3.tile([128, 512], bf16, tag="q3")
                for c in range(4):
                    nc.tensor.transpose(
                        q3[:, 128 * c:128 * c + 128],
                        t3[:, 128 * c:128 * c + 128], I128)
                Qs = qp.tile([128, 512], bf16, tag="qs")
                nc.vector.tensor_tensor(Qs[:], q3[:], Gt[:], OP.mult)

                for j in range(8):
                    s = 8 * t + j
                    nt_ = s // SEG
                    qcol = 128 * (j // 2) + 64 * (j % 2)
                    nc.tensor.matmul(
                        MSG[:, 64 * nt_:64 * nt_ + 64],
                        OHRt[:, 128 * j:128 * j + 128],
                        Qs[:, qcol:qcol + 64],
                        start=(s % SEG == 0), stop=(s % SEG == SEG - 1),
                        skip_group_check=True)

            # ---- epilogue ----
            MSUM = outp.tile([128, 8], f32)
            nc.vector.tensor_reduce(
                MSUM[:], MSG[:].rearrange("p (n h) -> p n h", h=64),
                mybir.AxisListType.X, OP.add)
            OUTT = outp.tile([128, 8], f32)
            nc.vector.scalar_tensor_tensor(
                OUTT[:], MSUM[:], 1.0 / AVG, CNODE, OP.mult, OP.add)
            nc.sync.dma_start(out_d[:], OUTT[:])

    nc.compile()
    return nc


def _host_prep(inputs):
    pos = np.asarray(inputs["positions"], np.float32)
    shifts = np.asarray(inputs["shifts"], np.float32)
    ei = np.asarray(inputs["edge_index"])
    species = np.asarray(inputs["species"]).astype(np.int64)
    ae = np.asarray(inputs["atomic_energies"], np.float32)
    w_embed = np.asarray(inputs["w_embed"], np.float32)
    w_up = np.asarray(inputs["w_up"], np.float32)
    W1 = np.asarray(inputs["W1"], np.float32)
    W2 = np.asarray(inputs["W2"], np.float32)
    W3 = np.asarray(inputs["W3"], np.float32)
    W4 = np.asarray(inputs["W4"], np.float32)
    w_lin = np.asarray(inputs["w_lin"], np.float32)
    w_skip = np.asarray(inputs["w_skip"], np.float32)
    w_sym = np.asarray(inputs["w_sym"], np.float32)
    w_lin2 = np.asarray(inputs["w_lin2"], np.float32)
    w_ro = np.asarray(inputs["w_readout"], np.float32)

    # collapsed weight tables
    hu = w_embed @ w_up                                   # [Z,K]
    alpha = w_lin2[0] @ w_ro                              # [K]
    delta = np.einsum("qk,zk,k->zq", w_lin[0], w_sym[0], alpha)  # [Z,K]
    W4_0 = np.ascontiguousarray(W4.reshape(64, K, 4)[:, :, 0])   # [64,K]
    Gamma = np.einsum("hk,sk,zk->szh", W4_0, hu, delta)   # [Z,Z,64]
    sct = np.einsum("zk,zkj->zj", w_embed, w_skip) / np.sqrt(Z)
    cz = ae + sct @ w_ro                                  # [Z]

    send, recv = ei[0].astype(np.int64), ei[1].astype(np.int64)
    vec = pos[recv] - pos[send] + shifts
    rsq = (vec * vec).sum(-1)
    keep = rsq < (R_MAX * R_MAX + 1e-3)
    vec = vec[keep]
    sp_s = species[send[keep]]
    recv = recv[keep]
    sp_r = species[recv]

    core = recv // NPC
    loc = recv % NPC
    ntile = loc // 128
    lrow = loc % 128

    order = np.lexsort((ntile, core))
    vec, sp_s, sp_r, lrow = vec[order], sp_s[order], sp_r[order], lrow[order]
    core, ntile = core[order], ntile[order]
    gid = core * NT + ntile
    counts = np.bincount(gid, minlength=NCORES * NT)
    SEG = int(np.ceil(counts.max() / 128))
    S = NT * SEG
    NBLK = S // 8

    VEa = np.zeros((NCORES, 128, S, 3), np.float32)
    VEa[:, :, :, 0] = 10.0  # pad edges: r=10 -> env masked to 0
    Ga = np.zeros((NCORES, NBLK, 128, 8, 64), BF16)
    OHa = np.zeros((NCORES, NBLK, 128, 8, 128), BF16)

    Gedge = Gamma[sp_s, sp_r].astype(BF16)   # [E, 64]

    starts = np.zeros(NCORES * NT + 1, np.int64)
    np.cumsum(counts, out=starts[1:])
    for c_ in range(NCORES):
        for t in range(NT):
            g = c_ * NT + t
            a, b = starts[g], starts[g + 1]
            n = b - a
            idx = np.arange(n)
            sub = t * SEG + idx // 128        # global subtile
            row = idx % 128
            blk = sub // 8
            jj = sub % 8
            VEa[c_, row, sub, :] = vec[a:b]
            Ga[c_, blk, row, jj, :] = Gedge[a:b]
            OHa[c_, blk, row, jj, lrow[a:b]] = 1.0

    # per-subtile-in-block j -> G column offset 128*(j//2)+64*(j%2)
    # reorder the j axis of Ga accordingly: cols = [j0h, j1h, j2h, j3h, ...]
    # layout [128, 8, 64] flat col = 64*j + h ; wanted col = 128*(j//2)+64*(j%2)+h
    # 64*j + h == 128*(j//2) + 64*(j%2) + h  -> identical!  (j = 2*(j//2)+j%2)

    cnode = np.zeros((NCORES, 128, 8), np.float32)
    for c_ in range(NCORES):
        spc = species[c_ * NPC:(c_ + 1) * NPC]
        czc = cz[spc]
        for t in range(NT):
            nloc = min(128, NPC - t * 128)
            cnode[c_, :nloc, t] = czc[t * 128:t * 128 + nloc]

    n_ = np.arange(1, NB + 1, dtype=np.float32)
    cb8 = n_ / (2.0 * R_MAX)    # th = cb*r ; sin(2*pi*th) = sin(n*pi*r/R)
    constf = np.zeros((NCORES, 128, 18), np.float32)
    constf[:, :, 0:8] = cb8[None, None, :]
    constf[:, :, 8:16] = cnode
    constf[:, :, 16] = np.full((1,), 1, np.int32).view(np.float32)[0]
    constf[:, :, 17] = np.full((1,), 0x5F3759DF, np.int32).view(np.float32)[0]

    # bf16 weight consts (same for all cores)
    cb = np.zeros((128, 896), np.float32)
    for j in range(4):
        q = np.zeros((64, 128), np.float32)
        q[8 * (2 * j):8 * (2 * j) + 8, 0:64] = W1
        q[8 * (2 * j + 1):8 * (2 * j + 1) + 8, 64:128] = W1
        cb[0:64, 128 * j:128 * j + 128] = q
    wbd = np.zeros((128, 128), np.float32)
    wbd[0:64, 0:64] = W2
    wbd[64:128, 64:128] = W2
    cb[:, 512:640] = wbd
    wbd = np.zeros((128, 128), np.float32)
    wbd[0:64, 0:64] = W3
    wbd[64:128, 64:128] = W3
    cb[:, 640:768] = wbd
    cb[:, 768:896] = np.eye(128, dtype=np.float32)
    constb = cb.astype(BF16)

    return SEG, VEa, Ga, OHa, constf, constb


def kernel(**inputs):
    global LAST_RESULTS
    from concourse.bass_utils import run_bass_kernel_spmd

    SEG, VEa, Ga, OHa, constf, constb = _host_prep(inputs)
    S = NT * SEG
    NBLK = S // 8
    if SEG not in _prog_cache:
        _prog_cache[SEG] = _build_program(SEG)
    nc = _prog_cache[SEG]

    in_maps = []
    for c_ in range(NCORES):
        m = {
            "ve": np.ascontiguousarray(VEa[c_].reshape(128, 3 * S)),
            "gtab": np.ascontiguousarray(Ga[c_].reshape(NBLK, 128, 512)),
            "ohr": np.ascontiguousarray(OHa[c_].reshape(NBLK, 128, 1024)),
            "constf": np.ascontiguousarray(constf[c_]),
            "constb": constb,
        }
        in_maps.append(m)

    res = run_bass_kernel_spmd(
        nc, in_maps, core_ids=list(range(NCORES)), trace=TRACE)
    LAST_RESULTS = res

    out = np.concatenate(
        [res.results[c_]["out"].T.reshape(1024)[:NPC] for c_ in range(NCORES)])
    return out.astype(np.float32)


# revision 15
# speedup vs baseline: 1.2904x; 1.2904x over previous
"""MACE-style GNN message passing on 8 Trainium2 NeuronCores.

Only the l=0 (scalar) channel of the reference reaches the output, so the
network collapses algebraically: per edge, the radial MLP's last hidden
t3 (64) is dotted with a per-(sender-species, receiver-species) vector
Gamma[s,z] = W4_0 @ (hu[s] * delta[z]), where hu = w_embed@w_up and
delta[z] folds w_lin[0], w_sym[0], w_lin2[0] and w_readout.  Node energy
is then ae[z]+beta[z] + (1/16) * scatter_sum(eps_e).

Device pipeline (bf16 matmuls, fp32 geometry):
  - batched DVE geometry for all edges: r via bit-trick rsqrt, envelope,
    bessel sine args; one ACT Sin; ef = sh*w (bf16)
  - per 1024-edge block: PE transpose ef -> [64 feat, e], 3-layer MLP
    (silu on ACT), PE transpose t3 -> [e, h], product with gathered
    Gamma rows (DVE/GPSIMD alternating), and a per-128-edge-subtile
    scatter matmul (one-hot stationary) accumulating msg[128 nodes, 64]
    per node tile in PSUM
  - epilogue: reduce h on DVE, scale + per-species constant, DMA out

Sharding: receivers range-partitioned (1000 nodes/core); per (core,
node-tile) edge groups padded to a uniform SEG subtiles of 128 so all
cores run one SPMD program.  Edges with r >= r_max are dropped on host.
"""

import sys
import numpy as np

sys.path.insert(0, "/opt/trn_rl_repo")

import ml_dtypes

BF16 = ml_dtypes.bfloat16

R_MAX = 5.0
EPS = 1e-9
AVG = 16.0
N_NODES = 8000
Z = 10
K = 128
NB = 8
NCORES = 8
NPC = N_NODES // NCORES       # nodes per core (1000)
NT = 8                        # node tiles per core (128 nodes each)

TRACE = False
LAST_RESULTS = None

_prog_cache = {}


def _build_program(SEG):
    """SPMD Bass program; SEG = 128-edge subtiles per 128-node tile."""
    from concourse import bass, bacc, mybir
    from concourse.tile import TileContext
    from contextlib import ExitStack

    f32 = mybir.dt.float32
    bf16 = mybir.dt.bfloat16
    i32 = mybir.dt.int32
    AF = mybir.ActivationFunctionType
    OP = mybir.AluOpType
    PSUM = bass.MemorySpace.PSUM

    S = NT * SEG              # total subtiles per core
    NBLK = S // 8             # 1024-edge blocks
    S3 = 3 * S
    S8 = 8 * S

    nc = bacc.Bacc(None, target_bir_lowering=False)

    ve_d = nc.dram_tensor("ve", [128, S3], f32, kind="ExternalInput")
    g_d = nc.dram_tensor("gtab", [NBLK, 128, 512], bf16, kind="ExternalInput")
    ohr_d = nc.dram_tensor("ohr", [NBLK, 128, 1024], bf16, kind="ExternalInput")
    cf_d = nc.dram_tensor("constf", [128, 18], f32, kind="ExternalInput")
    cb_d = nc.dram_tensor("constb", [128, 896], bf16, kind="ExternalInput")
    out_d = nc.dram_tensor("out", [128, 8], f32, kind="ExternalOutput")

    # geometry chunk boundaries (in blocks): chunk A computed upfront on
    # DVE; chunk B's ops are interleaved into early loop iterations so the
    # in-order DVE queue never hides per-block work behind a long blob.
    BLK_A = min(4, NBLK)

    with TileContext(nc) as tc:
        with ExitStack() as stack:
            cp = stack.enter_context(tc.tile_pool(name="const", bufs=1))
            geo = stack.enter_context(tc.tile_pool(name="geo", bufs=1))
            efsp = stack.enter_context(tc.tile_pool(name="efsp", bufs=3))
            gp = stack.enter_context(tc.tile_pool(name="gp", bufs=6))
            ohp = stack.enter_context(tc.tile_pool(name="ohp", bufs=6))
            t1p = stack.enter_context(tc.tile_pool(name="t1p", bufs=3))
            t2p = stack.enter_context(tc.tile_pool(name="t2p", bufs=3))
            t3p = stack.enter_context(tc.tile_pool(name="t3p", bufs=3))
            qp = stack.enter_context(tc.tile_pool(name="qp", bufs=3))
            outp = stack.enter_context(tc.tile_pool(name="outp", bufs=1))
            pefp = stack.enter_context(tc.tile_pool(name="pefp", bufs=2, space=PSUM))
            pml = stack.enter_context(tc.tile_pool(name="pml", bufs=3, space=PSUM))
            pq3 = stack.enter_context(tc.tile_pool(name="pq3", bufs=2, space=PSUM))
            pmsg = stack.enter_context(tc.tile_pool(name="pmsg", bufs=1, space=PSUM))

            # ---- constants ----
            CTF = cp.tile([128, 18], f32)
            nc.sync.dma_start(CTF[:], cf_d[:])
            CTB = cp.tile([128, 896], bf16)
            nc.sync.dma_start(CTB[:], cb_d[:])
            CB8 = CTF[:, 0:8]
            CNODE = CTF[:, 8:16]
            ONEI = CTF[:, 16:17].bitcast(i32)
            MAGIC = CTF[:, 17:18].bitcast(i32)
            W1P = [CTB[0:64, 128 * j:128 * j + 128] for j in range(4)]
            W2BD = CTB[:, 512:640]
            W3BD = CTB[:, 640:768]
            I128 = CTB[:, 768:896]

            VE = geo.tile([128, S3], f32)
            nc.sync.dma_start(VE[:], ve_d[:])

            tc.strict_bb_all_engine_barrier()

            def emit_geo(b0, b1):
                """Geometry for blocks [b0, b1): returns dict with the EFB
                tile, the DVE op thunks (in dependency order), and the
                trailing ACT sin / DVE ef-multiply thunks."""
                nb = b1 - b0
                sb = 8 * nb
                w8 = 64 * nb
                VEc = VE[:, 24 * b0:24 * b1]
                SQ = geo.tile([128, 3 * sb], f32)
                SC = geo.tile([128, 8 * sb], f32)
                TH = geo.tile([128, w8], f32)
                KI = geo.tile([128, w8], i32)
                KF = geo.tile([128, w8], f32)
                SA = geo.tile([128, w8], f32)
                GTt = geo.tile([128, w8], f32)
                SH = geo.tile([128, w8], f32)
                EFB = geo.tile([128, w8], bf16)

                def sl(i):
                    return SC[:, i * sb:(i + 1) * sb]

                ss, y, h2, q2, r_, x, u1, u2 = (sl(i) for i in range(8))
                y_i = y.bitcast(i32)
                V = nc.vector
                s2r = float(np.sqrt(2.0 / R_MAX))
                ops = [
                    lambda: V.tensor_tensor(SQ[:], VEc, VEc, OP.mult),
                    lambda: V.tensor_reduce(
                        ss, SQ[:].rearrange("p (s c) -> p s c", c=3),
                        mybir.AxisListType.X, OP.add),
                    lambda: V.tensor_scalar(ss, ss, EPS, None, OP.add),
                    lambda: V.tensor_tensor(
                        y_i, ss.bitcast(i32),
                        ONEI.broadcast_to([128, sb]), OP.arith_shift_right),
                    lambda: V.tensor_tensor(
                        y_i, MAGIC.broadcast_to([128, sb]), y_i, OP.subtract),
                    lambda: V.tensor_scalar(h2, ss, 0.5, None, OP.mult),
                ]
                for _ in range(2):
                    ops += [
                        lambda: V.tensor_tensor(q2, y, y, OP.mult),
                        lambda: V.tensor_tensor(q2, q2, h2, OP.mult),
                        lambda: V.tensor_scalar(q2, q2, -1.0, 1.5, OP.mult, OP.add),
                        lambda: V.tensor_tensor(y, y, q2, OP.mult),
                    ]
                ops += [
                    lambda: V.tensor_tensor(r_, ss, y, OP.mult),
                    lambda: V.tensor_scalar(x, r_, 1.0 / R_MAX, None, OP.mult),
                    lambda: V.tensor_tensor(u1, x, x, OP.mult),
                    lambda: V.tensor_tensor(u1, u1, u1, OP.mult),
                    lambda: V.tensor_tensor(u1, u1, x, OP.mult),
                    lambda: V.tensor_scalar(u2, x, -15.0, 35.0, OP.mult, OP.add),
                    lambda: V.tensor_tensor(u2, u2, x, OP.mult),
                    lambda: V.scalar_tensor_tensor(u1, u2, -21.0, u1, OP.add, OP.mult),
                    lambda: V.tensor_scalar(u2, x, 1.0, None, OP.is_lt),
                    lambda: V.scalar_tensor_tensor(u1, u1, 1.0, u2, OP.add, OP.mult),
                    lambda: V.scalar_tensor_tensor(u2, u1, s2r, y, OP.mult, OP.mult),
                    lambda: V.tensor_tensor(
                        TH[:].rearrange("p (s b) -> p s b", b=8),
                        CB8.unsqueeze(1).broadcast_to([128, sb, 8]),
                        r_.unsqueeze(2).broadcast_to([128, sb, 8]), OP.mult),
                    lambda: V.tensor_copy(KI[:], TH[:]),
                    lambda: V.tensor_copy(KF[:], KI[:]),
                    lambda: V.tensor_tensor(SA[:], TH[:], KF[:], OP.subtract),
                    lambda: V.tensor_scalar(GTt[:], SA[:], 0.5, None, OP.is_gt),
                    lambda: V.scalar_tensor_tensor(
                        SA[:], GTt[:], -1.0, SA[:], OP.mult, OP.add),
                ]
                sin_op = lambda: nc.scalar.activation(
                    SH[:], SA[:], AF.Sin, scale=float(2 * np.pi))
                efb_op = lambda: V.tensor_tensor(
                    EFB[:].rearrange("p (s b) -> p s b", b=8),
                    SH[:].rearrange("p (s b) -> p s b", b=8),
                    u2.unsqueeze(2).broadcast_to([128, sb, 8]), OP.mult)
                return {"EFB": EFB, "ops": ops, "sin": sin_op, "efb": efb_op}

            gA = emit_geo(0, BLK_A)
            for op in gA["ops"]:
                op()
            gA["sin"]()
            gA["efb"]()
            gB = emit_geo(BLK_A, NBLK) if BLK_A < NBLK else None
            NSL = 3  # drain chunk-B DVE ops over iterations 0..NSL-1

            # ---- software-pipelined block loop ----
            # iteration i issues: efT(i), L1(i-1), L2(i-2), L3(i-3),
            # t3T+product(i-4), scatter(i-5)
            MSG = pmsg.tile([128, 512], f32, tag="msg")
            efs = {}
            t1s = {}
            t2s = {}
            t3s = {}
            qss = {}
            gts = {}
            ohrs = {}
            NITER = NBLK + 5
            for i in range(NITER):
                if i < NBLK:
                    gts[i] = gp.tile([128, 512], bf16, tag="gt", name="gt")
                    nc.sync.dma_start(gts[i][:], g_d[i])
                    ohrs[i] = ohp.tile([128, 1024], bf16, tag="ohr", name="ohrt")
                    nc.sync.dma_start(ohrs[i][:], ohr_d[i])

                # stage 1: ef transpose (PE) + copy to SBUF (DVE)
                if i < NBLK:
                    if i < BLK_A:
                        EFBc, off = gA["EFB"], i
                    else:
                        EFBc, off = gB["EFB"], i - BLK_A
                    pef = pefp.tile([64, 128], bf16, tag="pef")
                    nc.tensor.transpose(
                        pef[:], EFBc[:, 64 * off:64 * off + 64], I128)
                    efs[i] = efsp.tile([64, 128], bf16, tag="efs", name="efs")
                    nc.vector.tensor_copy(efs[i][:], pef[:])

                # interleave deferred chunk-B geometry behind the efcopy
                if gB is not None and i < NSL:
                    a_ = i * len(gB["ops"]) // NSL
                    b_ = (i + 1) * len(gB["ops"]) // NSL
                    for op in gB["ops"][a_:b_]:
                        op()
                if gB is not None and i == NSL:
                    gB["sin"]()
                    gB["efb"]()

                # stage 2: L1 (PE) + silu1 (ACT)
                j = i - 1
                if 0 <= j < NBLK:
                    p1 = pml.tile([128, 512], f32, tag="pml")
                    for k in range(4):
                        nc.tensor.matmul(p1[:, 128 * k:128 * k + 128],
                                         W1P[k], efs[j][:],
                                         start=True, stop=True)
                    t1s[j] = t1p.tile([128, 512], bf16, tag="t1", name="t1")
                    nc.scalar.activation(t1s[j][:], p1[:], AF.Silu)
                    del efs[j]

                # stage 3: L2 + silu2
                j = i - 2
                if 0 <= j < NBLK:
                    p2 = pml.tile([128, 512], f32, tag="pml")
                    nc.tensor.matmul(p2[:], W2BD, t1s[j][:],
                                     start=True, stop=True)
                    t2s[j] = t2p.tile([128, 512], bf16, tag="t2", name="t2")
                    nc.scalar.activation(t2s[j][:], p2[:], AF.Silu)
                    del t1s[j]

                # stage 4: L3 + silu3
                j = i - 3
                if 0 <= j < NBLK:
                    p3 = pml.tile([128, 512], f32, tag="pml")
                    nc.tensor.matmul(p3[:], W3BD, t2s[j][:],
                                     start=True, stop=True)
                    t3s[j] = t3p.tile([128, 512], bf16, tag="t3", name="t3")
                    nc.scalar.activation(t3s[j][:], p3[:], AF.Silu)
                    del t2s[j]

                # stage 5: t3 transpose (PE) + product (DVE)
                j = i - 4
                if 0 <= j < NBLK:
                    q3 = pq3.tile([128, 512], bf16, tag="q3")
                    for c in range(4):
                        nc.tensor.transpose(
                            q3[:, 128 * c:128 * c + 128],
                            t3s[j][:, 128 * c:128 * c + 128], I128)
                    qss[j] = qp.tile([128, 512], bf16, tag="qs", name="qs")
                    nc.vector.tensor_tensor(qss[j][:], q3[:], gts[j][:],
                                            OP.mult)
                    del t3s[j], gts[j]

                # stage 6: scatter (PE)
                j = i - 5
                if 0 <= j < NBLK:
                    for k in range(8):
                        s = 8 * j + k
                        nt_ = s // SEG
                        qcol = 128 * (k // 2) + 64 * (k % 2)
                        nc.tensor.matmul(
                            MSG[:, 64 * nt_:64 * nt_ + 64],
                            ohrs[j][:, 128 * k:128 * k + 128],
                            qss[j][:, qcol:qcol + 64],
                            start=(s % SEG == 0), stop=(s % SEG == SEG - 1),
                            skip_group_check=True)
                    del qss[j], ohrs[j]

            # ---- epilogue ----
            MSUM = outp.tile([128, 8], f32)
            nc.vector.tensor_reduce(
                MSUM[:], MSG[:].rearrange("p (n h) -> p n h", h=64),
                mybir.AxisListType.X, OP.add)
            OUTT = outp.tile([128, 8], f32)
            nc.vector.scalar_tensor_tensor(
                OUTT[:], MSUM[:], 1.0 / AVG, CNODE, OP.mult, OP.add)
            nc.sync.dma_start(out_d[:], OUTT[:])

    nc.compile()
    return nc


def _host_prep(inputs):
    pos = np.asarray(inputs["positions"], np.float32)
    shifts = np.asarray(inputs["shifts"], np.float32)
    ei = np.asarray(inputs["edge_index"])
    species = np.asarray(inputs["species"]).astype(np.int64)
    ae = np.asarray(inputs["atomic_energies"], np.float32)
    w_embed = np.asarray(inputs["w_embed"], np.float32)
    w_up = np.asarray(inputs["w_up"], np.float32)
    W1 = np.asarray(inputs["W1"], np.float32)
    W2 = np.asarray(inputs["W2"], np.float32)
    W3 = np.asarray(inputs["W3"], np.float32)
    W4 = np.asarray(inputs["W4"], np.float32)
    w_lin = np.asarray(inputs["w_lin"], np.float32)
    w_skip = np.asarray(inputs["w_skip"], np.float32)
    w_sym = np.asarray(inputs["w_sym"], np.float32)
    w_lin2 = np.asarray(inputs["w_lin2"], np.float32)
    w_ro = np.asarray(inputs["w_readout"], np.float32)

    # collapsed weight tables
    hu = w_embed @ w_up                                   # [Z,K]
    alpha = w_lin2[0] @ w_ro                              # [K]
    delta = np.einsum("qk,zk,k->zq", w_lin[0], w_sym[0], alpha)  # [Z,K]
    W4_0 = np.ascontiguousarray(W4.reshape(64, K, 4)[:, :, 0])   # [64,K]
    Gamma = np.einsum("hk,sk,zk->szh", W4_0, hu, delta)   # [Z,Z,64]
    sct = np.einsum("zk,zkj->zj", w_embed, w_skip) / np.sqrt(Z)
    cz = ae + sct @ w_ro                                  # [Z]

    send, recv = ei[0].astype(np.int64), ei[1].astype(np.int64)
    vec = pos[recv] - pos[send] + shifts
    rsq = (vec * vec).sum(-1)
    keep = rsq < (R_MAX * R_MAX + 1e-3)
    vec = vec[keep]
    sp_s = species[send[keep]]
    recv = recv[keep]
    sp_r = species[recv]

    core = recv // NPC
    loc = recv % NPC
    ntile = loc // 128
    lrow = loc % 128

    order = np.lexsort((ntile, core))
    vec, sp_s, sp_r, lrow = vec[order], sp_s[order], sp_r[order], lrow[order]
    core, ntile = core[order], ntile[order]
    gid = core * NT + ntile
    counts = np.bincount(gid, minlength=NCORES * NT)
    SEG = int(np.ceil(counts.max() / 128))
    S = NT * SEG
    NBLK = S // 8

    VEa = np.zeros((NCORES, 128, S, 3), np.float32)
    VEa[:, :, :, 0] = 10.0  # pad edges: r=10 -> env masked to 0
    Ga = np.zeros((NCORES, NBLK, 128, 8, 64), BF16)
    OHa = np.zeros((NCORES, NBLK, 128, 8, 128), BF16)

    Gedge = Gamma[sp_s, sp_r].astype(BF16)   # [E, 64]

    starts = np.zeros(NCORES * NT + 1, np.int64)
    np.cumsum(counts, out=starts[1:])
    for c_ in range(NCORES):
        for t in range(NT):
            g = c_ * NT + t
            a, b = starts[g], starts[g + 1]
            n = b - a
            idx = np.arange(n)
            sub = t * SEG + idx // 128        # global subtile
            row = idx % 128
            blk = sub // 8
            jj = sub % 8
            VEa[c_, row, sub, :] = vec[a:b]
            Ga[c_, blk, row, jj, :] = Gedge[a:b]
            OHa[c_, blk, row, jj, lrow[a:b]] = 1.0

    # per-subtile-in-block j -> G column offset 128*(j//2)+64*(j%2)
    # reorder the j axis of Ga accordingly: cols = [j0h, j1h, j2h, j3h, ...]
    # layout [128, 8, 64] flat col = 64*j + h ; wanted col = 128*(j//2)+64*(j%2)+h
    # 64*j + h == 128*(j//2) + 64*(j%2) + h  -> identical!  (j = 2*(j//2)+j%2)

    cnode = np.zeros((NCORES, 128, 8), np.float32)
    for c_ in range(NCORES):
        spc = species[c_ * NPC:(c_ + 1) * NPC]
        czc = cz[spc]
        for t in range(NT):
            nloc = min(128, NPC - t * 128)
            cnode[c_, :nloc, t] = czc[t * 128:t * 128 + nloc]

    n_ = np.arange(1, NB + 1, dtype=np.float32)
    cb8 = n_ / (2.0 * R_MAX)    # th = cb*r ; sin(2*pi*th) = sin(n*pi*r/R)
    constf = np.zeros((NCORES, 128, 18), np.float32)
    constf[:, :, 0:8] = cb8[None, None, :]
    constf[:, :, 8:16] = cnode
    constf[:, :, 16] = np.full((1,), 1, np.int32).view(np.float32)[0]
    constf[:, :, 17] = np.full((1,), 0x5F3759DF, np.int32).view(np.float32)[0]

    # bf16 weight consts (same for all cores)
    cb = np.zeros((128, 896), np.float32)
    for j in range(4):
        q = np.zeros((64, 128), np.float32)
        q[8 * (2 * j):8 * (2 * j) + 8, 0:64] = W1
        q[8 * (2 * j + 1):8 * (2 * j + 1) + 8, 64:128] = W1
        cb[0:64, 128 * j:128 * j + 128] = q
    wbd = np.zeros((128, 128), np.float32)
    wbd[0:64, 0:64] = W2
    wbd[64:128, 64:128] = W2
    cb[:, 512:640] = wbd
    wbd = np.zeros((128, 128), np.float32)
    wbd[0:64, 0:64] = W3
    wbd[64:128, 64:128] = W3
    cb[:, 640:768] = wbd
    cb[:, 768:896] = np.eye(128, dtype=np.float32)
    constb = cb.astype(BF16)

    return SEG, VEa, Ga, OHa, constf, constb


def kernel(**inputs):
    global LAST_RESULTS
    from concourse.bass_utils import run_bass_kernel_spmd

    SEG, VEa, Ga, OHa, constf, constb = _host_prep(inputs)
    S = NT * SEG
    NBLK = S // 8
    if SEG not in _prog_cache:
        _prog_cache[SEG] = _build_program(SEG)
    nc = _prog_cache[SEG]

    in_maps = []
    for c_ in range(NCORES):
        m = {
            "ve": np.ascontiguousarray(VEa[c_].reshape(128, 3 * S)),
            "gtab": np.ascontiguousarray(Ga[c_].reshape(NBLK, 128, 512)),
            "ohr": np.ascontiguousarray(OHa[c_].reshape(NBLK, 128, 1024)),
            "constf": np.ascontiguousarray(constf[c_]),
            "constb": constb,
        }
        in_maps.append(m)

    res = run_bass_kernel_spmd(
        nc, in_maps, core_ids=list(range(NCORES)), trace=TRACE)
    LAST_RESULTS = res

    out = np.concatenate(
        [res.results[c_]["out"].T.reshape(1024)[:NPC] for c_ in range(NCORES)])
    return out.astype(np.float32)
